# revision 28
# baseline (speedup 1.0000x reference)
"""Trainium2 Bass kernel: Anscombe transform -> 3x3 Gaussian blur -> inverse
Anscombe, on a [1,4096,4096,3] fp32 image, sharded over H across 8 NeuronCores.

Pipeline per core (512 output rows, input slab of 514 rows incl. 1-row halo):
  DMA in -> ACT: at = sqrt((4x+1.5)/LAM^2)   (pad rows hold -0.375 so at=0,
  matching the reference's zero padding; the 1/LAM fold makes the PSUM hold
  yt = y/LAM, which reciprocal_approx_fast turns into r = LAM/y)
  -> PE: separable 3x3 conv as accumulated matmuls per 512-col PSUM chunk
  (vertical via banded weight matrix over partitions, horizontal via free-dim
  shifts of +-3 interleaved channels)
  -> ACT: u = Square(0.5*LAM*yt) = 0.25*y^2
  -> DVE: r = reciprocal_approx_fast(yt) = LAM/y
  -> DVE: custom op  out = r^2*(CT*r+BT) + ((CT*r - 0.125) + u)
          == 0.25 y^2 - 0.125 + a/y + b/y^2 + c/y^3
  -> DMA out.

Matmul precision modes (mm_mode):
  "bf16x3" (default): 3-term compensated bf16 split  w*at ~= wh*ah + wl*ah
     + wh*al  with ah=bf16(at), al=at-ah (computed on the idle GpSimd
     engine), wh=bf16(w), wl=w-wh. ~1e-5 scale-relative output error,
     full-speed PE (1 cycle/row).
  "fp32r": single fp32r (rounded-operand) matmuls; ~4e-4 scale-relative
     error, 3x fewer matmuls.
  "fp32": exact but 4 cycles/row on the PE (4-5x slower overall).
"""

import numpy as np
import ml_dtypes

import concourse.bass as bass
import concourse.bacc as bacc
import concourse.mybir as mybir
import concourse.tile as tile
from concourse import dve_ops
from concourse.bass_utils import run_bass_kernel_spmd
from concourse.dve_spec import C0, C1, C2, Spec, Src0, Src1, _has_src1, lower as dve_lower, sq
from concourse.dve_uop import DveOpSpec

F32 = mybir.dt.float32
F32R = mybir.dt.float32r
BF16 = mybir.dt.bfloat16
FP16 = mybir.dt.float16

# ---------------------------------------------------------------- constants
H, W, CH = 4096, 4096, 3
WC = W * CH
N_CORES = 8
H_CORE = H // N_CORES          # output rows per core
BLOCK = 126                    # max output rows per block (128 input rows)
CHUNK = 512                    # matmul N (one PSUM bank)
GROUP = 2048                   # postprocess tile width (4 PSUM banks)
PAD_VAL = -0.375               # sqrt affine maps this to exactly 0

# Gaussian kernel exactly as the reference builds it (fp32 throughout)
_coords = np.arange(-1, 2, dtype=np.float32)
_g = np.exp(-(_coords[:, None] ** 2 + _coords[None, :] ** 2)
            / (np.float32(2.0) * np.float32(1.3) ** 2)).astype(np.float32)
K2D = (_g / _g.sum()).astype(np.float32)       # [3,3], rows=dy, cols=dx

_s15 = np.sqrt(np.float64(1.5))
A_C = 0.25 * _s15              # coefficient of 1/y
B_C = np.float64(-11.0 / 8.0)  # coefficient of 1/y^2
C_C = 0.625 * _s15             # coefficient of 1/y^3
LAM = float(np.sqrt(C_C / A_C))          # sqrt(2.5); yt = y/LAM, r = LAM/y
CT = float(C_C / LAM ** 3)
BT = float(B_C / LAM ** 2)
SQ_SCALE = float(0.5 * LAM)              # Square(SQ_SCALE*yt) = 0.25*y^2
SQRT_SCALE = float(4.0 / LAM ** 2)       # at = sqrt(SQRT_SCALE*x + SQRT_BIAS)
SQRT_BIAS = float(1.5 / LAM ** 2)        #    = sqrt(4x+1.5)/LAM

MM_MODE = "fp32r"              # "fp32r" | "bf16x3" | "fp32"
SPLIT_ON_GPSIMD = True         # False routes the h/l split to the DVE


# ------------------------------------------------- custom DVE op (the tail)
def _register_tail_op():
    """out = sq(r)*(CT*r + BT) + ((CT*r - 0.125) + u); r=Src0, u=Src1."""
    name = "ANSCOMBE_TAIL_ANT"
    for op in dve_ops.OPS:
        if op.name == name:
            return op
    h = Src0 * C0
    spec = Spec(
        body=sq(Src0) * (h + C1) + ((h + C2) + Src1),
        reference=lambda in0, in1, c0, c1, c2: (
            (in0.astype(np.float32) * in0) * (in0 * np.float32(c0) + np.float32(c1))
            + ((in0 * np.float32(c0) + np.float32(c2)) + in1)
        ).astype(np.float32),
    )
    row = max(dve_ops._SUB_OPCODE_FOR_NAME.values()) + 1
    assert row < 0x20
    dve_ops._SUB_OPCODE_FOR_NAME[name] = row
    shas = {}
    for ver in ("v3", "v4"):
        ds = DveOpSpec(name=name, opcode=row, uops=dve_lower(spec, ver=ver),
                       rd1_en=_has_src1(spec))
        shas[ver] = ds.sha(ver)
    op = dve_ops.DveOp(name, spec, subdim=False, uops_sha=shas)
    dve_ops.OPS.append(op)
    dve_ops.CUSTOM_DVE_SPECS[name] = spec
    return op


def _band(vals_by_tap, n_seg, dtype):
    """[128, n_seg*BLOCK] band matrices: seg j has vals_by_tap[j][d] on
    diagonal k-m = d (d in 0..2)."""
    wm = np.zeros((128, n_seg * BLOCK), dtype=np.float64)
    for j in range(n_seg):
        for d in range(3):
            col = vals_by_tap[j][d]
            for m in range(BLOCK):
                k = m + d
                if k < 128:
                    wm[k, j * BLOCK + m] = col
    return wm.astype(dtype)


def _weight_matrices(mm_mode):
    # tap j reads at columns shifted by 3*(j-1); vertical tap d = k-m.
    # K2D[d, j] is the (dy=d-1, dx=j-1) kernel value.
    w = K2D.astype(np.float64)  # [d, j]
    if mm_mode == "bf16x3":
        wh = w.astype(ml_dtypes.bfloat16).astype(np.float64)
        wl = w - wh
        taps = [[wh[d, j] for d in range(3)] for j in range(3)] + \
               [[wl[d, j] for d in range(3)] for j in range(3)]
        # bf16 wh values are exactly fp16-representable; the fp16 copy pairs
        # with the fp16 data-residual matmul.
        taps_h = [[wh[d, j] for d in range(3)] for j in range(3)]
        return _band(taps, 6, ml_dtypes.bfloat16), _band(taps_h, 3, np.float16)
    taps = [[w[d, j] for d in range(3)] for j in range(3)]
    return _band(taps, 3, np.float32), None


# ------------------------------------------------------------- bass program
def build_nc(h_out=H_CORE, wc=WC, mm_mode=MM_MODE):
    tail_op = _register_tail_op()
    h_in = h_out + 2
    nc = bacc.Bacc(None, target_bir_lowering=False)
    # const AP for the sqrt bias (activation converts float bias to an AP)
    _bias = nc.alloc_sbuf_tensor("const-sqrt-bias", [128, 1], F32)
    nc.gpsimd.memset(_bias.ap(), SQRT_BIAS)
    nc.const_aps.aps[(F32, SQRT_BIAS)] = _bias.ap()
    nc.all_engine_barrier()

    n_seg = 6 if mm_mode == "bf16x3" else 3
    w_dt = BF16 if mm_mode == "bf16x3" else F32
    mm_dt = {"bf16x3": BF16, "fp32r": F32R, "fp32": F32}[mm_mode]
    at_dt = F32 if mm_mode == "bf16x3" else mm_dt

    x = nc.declare_dram_parameter("x", [h_in, wc], F32, isOutput=False)
    wmat = nc.declare_dram_parameter("wm", [128, n_seg * BLOCK], w_dt, isOutput=False)
    if mm_mode == "bf16x3":
        wmath = nc.declare_dram_parameter("wmh", [128, 3 * BLOCK], FP16, isOutput=False)
    out = nc.declare_dram_parameter("out", [h_out, wc], F32, isOutput=True)

    # equal blocks (uniform matmul tile_size)
    n_blk = -(-h_out // BLOCK)
    base, rem = divmod(h_out, n_blk)
    blocks = []
    r0 = 0
    for i in range(n_blk):
        m = base + (1 if i < rem else 0)
        blocks.append((r0, m))
        r0 += m

    with tile.TileContext(nc) as tc:
        with (
            tc.tile_pool(name="consts", bufs=1) as cpool,
            tc.tile_pool(name="at", bufs=2) as atpool,
            tc.tile_pool(name="xpool", bufs=2) as xpool,
            tc.tile_pool(name="hpool", bufs=2) as hpool,
            tc.tile_pool(name="lpool", bufs=2) as lpool,
            tc.tile_pool(name="upool", bufs=2) as upool,
            tc.tile_pool(name="rpool", bufs=2) as rpool,
            tc.tile_pool(name="opool", bufs=3) as opool,
            tc.tile_pool(name="psum", bufs=2, space="PSUM") as pspool,
        ):
            if mm_mode in ("fp32r", "fp32"):
                # funnel weight deps through ACT (fp32 LDW has few wait slots)
                wt0 = cpool.tile([128, n_seg * BLOCK], F32)
                wt = cpool.tile([128, n_seg * BLOCK], mm_dt)
                nc.sync.dma_start(wt0[:], wmat[:])
                nc.scalar.activation(wt[:], wt0[:],
                                     mybir.ActivationFunctionType.Copy,
                                     bias=0.0, scale=1.0)
            else:
                wt = cpool.tile([128, n_seg * BLOCK], w_dt)
                nc.sync.dma_start(wt[:], wmat[:])
                wth = cpool.tile([128, 3 * BLOCK], FP16)
                nc.sync.dma_start(wth[:], wmath[:])

            for (r0, m) in blocks:
                k_in = m + 2
                at = atpool.tile([128, wc + 6], at_dt, tag="at")
                # zero 3-col borders (ACT writes -> valid fp32r producer too)
                for sl in (slice(0, 3), slice(wc + 3, wc + 6)):
                    nc.scalar.activation(at[:k_in, sl],
                                         nc.const_aps.tensor(0.0, (k_in, 3)),
                                         mybir.ActivationFunctionType.Copy,
                                         bias=0.0, scale=1.0)
                for g0 in range(0, wc, GROUP):
                    gw = min(GROUP, wc - g0)
                    xc = xpool.tile([128, GROUP], F32, tag="xc")
                    nc.sync.dma_start(xc[:k_in, :gw], x[r0:r0 + k_in, g0:g0 + gw])
                    nc.scalar.activation(at[:k_in, 3 + g0:3 + g0 + gw],
                                         xc[:k_in, :gw],
                                         mybir.ActivationFunctionType.Sqrt,
                                         bias=SQRT_BIAS, scale=SQRT_SCALE)
                for g0 in range(0, wc, GROUP):
                    gw = min(GROUP, wc - g0)
                    ps = pspool.tile([126, GROUP], F32, tag="ps")
                    if mm_mode == "bf16x3":
                        hh = hpool.tile([128, GROUP + 6], BF16, tag="hh")
                        ll = lpool.tile([128, GROUP + 6], FP16, tag="ll")
                        SPLIT_ENG = nc.gpsimd if SPLIT_ON_GPSIMD else nc.vector
                        SPLIT_ENG.tensor_copy(hh[:k_in, :gw + 6],
                                              at[:k_in, g0:g0 + gw + 6])
                        SPLIT_ENG.tensor_sub(ll[:k_in, :gw + 6],
                                             at[:k_in, g0:g0 + gw + 6],
                                             hh[:k_in, :gw + 6])
                        terms = [(0, wt, hh), (3, wt, hh), (0, wth, ll)]
                    else:
                        terms = [(0, wt, at)]
                    n_terms = 3 * len(terms)
                    for n0 in range(0, gw, CHUNK):
                        cw = min(CHUNK, gw - n0)
                        t = 0
                        for j in range(3):  # taps dx=-1,0,+1 (3*j col offset)
                            for (seg_off, wtile, rhs) in terms:
                                col = (n0 if rhs is not at else g0 + n0) + 3 * j
                                seg = (seg_off + j) * BLOCK
                                nc.tensor.matmul(
                                    ps[:m, n0:n0 + cw],
                                    wtile[:k_in, seg:seg + m],
                                    rhs[:k_in, col:col + cw],
                                    start=(t == 0), stop=(t == n_terms - 1),
                                )
                                t += 1
                    u = upool.tile([126, GROUP], F32, tag="u")
                    r = rpool.tile([126, GROUP], F32, tag="r")
                    o = opool.tile([126, GROUP], F32, tag="o")
                    nc.scalar.activation(u[:m, :gw], ps[:m, :gw],
                                         mybir.ActivationFunctionType.Square,
                                         scale=SQ_SCALE)
                    nc.vector.reciprocal_approx_fast(out=r[:m, :gw], in_=ps[:m, :gw])
                    nc.vector._custom_dve(tail_op, out=o[:m, :gw],
                                          in0=r[:m, :gw], in1=u[:m, :gw],
                                          s0=CT, s1=BT, imm2=-0.125)
                    nc.gpsimd.dma_start(out[r0:r0 + m, g0:g0 + gw], o[:m, :gw])
    nc.compile()
    return nc


# ------------------------------------------------------------------- driver
_CACHE = {}


def _get_nc(h_out, wc, mm_mode):
    key = (h_out, wc, mm_mode)
    if key not in _CACHE:
        _CACHE[key] = build_nc(h_out, wc, mm_mode)
    return _CACHE[key]


def run_sharded(x2d, n_cores=N_CORES, mm_mode=MM_MODE, trace=False, **kw):
    """x2d: [H, W*C] fp32 full image (2D). Returns ([H, W*C] fp32, results)."""
    h, wc = x2d.shape
    h_core = h // n_cores
    nc = _get_nc(h_core, wc, mm_mode)
    wm, wmh = _weight_matrices(mm_mode)
    in_maps = []
    for i in range(n_cores):
        lo, hi = i * h_core - 1, (i + 1) * h_core + 1
        src_lo, src_hi = max(lo, 0), min(hi, h)
        if lo < 0 or hi > h:
            slab = np.full((h_core + 2, wc), PAD_VAL, dtype=np.float32)
        else:
            slab = np.empty((h_core + 2, wc), dtype=np.float32)
        slab[src_lo - lo:src_hi - lo] = x2d[src_lo:src_hi]
        im_map = {"x": slab, "wm": wm}
        if wmh is not None:
            im_map["wmh"] = wmh
        in_maps.append(im_map)
    res = run_bass_kernel_spmd(nc, in_maps, list(range(n_cores)), trace=trace, **kw)
    full = np.concatenate([res.results[i]["out"] for i in range(n_cores)], axis=0)
    return full, res


def kernel(im: np.ndarray) -> np.ndarray:
    x2d = np.asarray(im, dtype=np.float32).reshape(H, WC)
    full, _ = run_sharded(x2d)
    return full.reshape(H, W, CH)


# revision 29
# speedup vs baseline: 1.0695x; 1.0695x over previous
"""Trainium2 Bass kernel: Anscombe transform -> 3x3 Gaussian blur -> inverse
Anscombe, on a [1,4096,4096,3] fp32 image, sharded over H across 8 NeuronCores.

Pipeline per core (512 output rows, input slab of 514 rows incl. 1-row halo):
  DMA in -> ACT: at = sqrt((4x+1.5)/LAM^2)   (pad rows hold -0.375 so at=0,
  matching the reference's zero padding; the 1/LAM fold makes the PSUM hold
  yt = y/LAM, which reciprocal_approx_fast turns into r = LAM/y)
  -> PE: separable 3x3 conv as accumulated matmuls per 512-col PSUM chunk
  (vertical via banded weight matrix over partitions, horizontal via free-dim
  shifts of +-3 interleaved channels)
  -> ACT: u = Square(0.5*LAM*yt) = 0.25*y^2
  -> DVE: r = reciprocal_approx_fast(yt) = LAM/y
  -> DVE: custom op  out = r^2*(CT*r+BT) + ((CT*r - 0.125) + u)
          == 0.25 y^2 - 0.125 + a/y + b/y^2 + c/y^3
  -> DMA out.

Matmul precision modes (mm_mode):
  "bf16x3" (default): 3-term compensated bf16 split  w*at ~= wh*ah + wl*ah
     + wh*al  with ah=bf16(at), al=at-ah (computed on the idle GpSimd
     engine), wh=bf16(w), wl=w-wh. ~1e-5 scale-relative output error,
     full-speed PE (1 cycle/row).
  "fp32r": single fp32r (rounded-operand) matmuls; ~4e-4 scale-relative
     error, 3x fewer matmuls.
  "fp32": exact but 4 cycles/row on the PE (4-5x slower overall).
"""

import numpy as np
import ml_dtypes

import concourse.bass as bass
import concourse.bacc as bacc
import concourse.mybir as mybir
import concourse.tile as tile
from concourse import dve_ops
from concourse.bass_utils import run_bass_kernel_spmd
from concourse.dve_spec import C0, C1, C2, Spec, Src0, Src1, _has_src1, lower as dve_lower, sq
from concourse.dve_uop import DveOpSpec

F32 = mybir.dt.float32
F32R = mybir.dt.float32r
BF16 = mybir.dt.bfloat16
FP16 = mybir.dt.float16

# ---------------------------------------------------------------- constants
H, W, CH = 4096, 4096, 3
WC = W * CH
N_CORES = 8
H_CORE = H // N_CORES          # output rows per core
BLOCK = 126                    # max output rows per block (128 input rows)
CHUNK = 512                    # matmul N (one PSUM bank)
GROUP = 2048                   # postprocess tile width (4 PSUM banks)
PAD_VAL = -0.375               # sqrt affine maps this to exactly 0

# Gaussian kernel exactly as the reference builds it (fp32 throughout)
_coords = np.arange(-1, 2, dtype=np.float32)
_g = np.exp(-(_coords[:, None] ** 2 + _coords[None, :] ** 2)
            / (np.float32(2.0) * np.float32(1.3) ** 2)).astype(np.float32)
K2D = (_g / _g.sum()).astype(np.float32)       # [3,3], rows=dy, cols=dx

_s15 = np.sqrt(np.float64(1.5))
A_C = 0.25 * _s15              # coefficient of 1/y
B_C = np.float64(-11.0 / 8.0)  # coefficient of 1/y^2
C_C = 0.625 * _s15             # coefficient of 1/y^3
LAM = float(np.sqrt(C_C / A_C))          # sqrt(2.5); yt = y/LAM, r = LAM/y
CT = float(C_C / LAM ** 3)
BT = float(B_C / LAM ** 2)
SQ_SCALE = float(0.5 * LAM)              # Square(SQ_SCALE*yt) = 0.25*y^2
SQRT_SCALE = float(4.0 / LAM ** 2)       # at = sqrt(SQRT_SCALE*x + SQRT_BIAS)
SQRT_BIAS = float(1.5 / LAM ** 2)        #    = sqrt(4x+1.5)/LAM

MM_MODE = "fp32r"              # "fp32r" | "bf16x3" | "fp32"
SPLIT_ON_GPSIMD = True         # False routes the h/l split to the DVE


# ------------------------------------------------- custom DVE op (the tail)
def _register_tail_op():
    """out = sq(r)*(CT*r + BT) + ((CT*r - 0.125) + u); r=Src0, u=Src1."""
    name = "ANSCOMBE_TAIL_ANT"
    for op in dve_ops.OPS:
        if op.name == name:
            return op
    h = Src0 * C0
    spec = Spec(
        body=sq(Src0) * (h + C1) + ((h + C2) + Src1),
        reference=lambda in0, in1, c0, c1, c2: (
            (in0.astype(np.float32) * in0) * (in0 * np.float32(c0) + np.float32(c1))
            + ((in0 * np.float32(c0) + np.float32(c2)) + in1)
        ).astype(np.float32),
    )
    row = max(dve_ops._SUB_OPCODE_FOR_NAME.values()) + 1
    assert row < 0x20
    dve_ops._SUB_OPCODE_FOR_NAME[name] = row
    shas = {}
    for ver in ("v3", "v4"):
        ds = DveOpSpec(name=name, opcode=row, uops=dve_lower(spec, ver=ver),
                       rd1_en=_has_src1(spec))
        shas[ver] = ds.sha(ver)
    op = dve_ops.DveOp(name, spec, subdim=False, uops_sha=shas)
    dve_ops.OPS.append(op)
    dve_ops.CUSTOM_DVE_SPECS[name] = spec
    return op


def _band(vals_by_tap, n_seg, dtype):
    """[128, n_seg*BLOCK] band matrices: seg j has vals_by_tap[j][d] on
    diagonal k-m = d (d in 0..2)."""
    wm = np.zeros((128, n_seg * BLOCK), dtype=np.float64)
    for j in range(n_seg):
        for d in range(3):
            col = vals_by_tap[j][d]
            for m in range(BLOCK):
                k = m + d
                if k < 128:
                    wm[k, j * BLOCK + m] = col
    return wm.astype(dtype)


def _weight_matrices(mm_mode):
    # tap j reads at columns shifted by 3*(j-1); vertical tap d = k-m.
    # K2D[d, j] is the (dy=d-1, dx=j-1) kernel value.
    w = K2D.astype(np.float64)  # [d, j]
    if mm_mode == "bf16x3":
        wh = w.astype(ml_dtypes.bfloat16).astype(np.float64)
        wl = w - wh
        taps = [[wh[d, j] for d in range(3)] for j in range(3)] + \
               [[wl[d, j] for d in range(3)] for j in range(3)]
        # bf16 wh values are exactly fp16-representable; the fp16 copy pairs
        # with the fp16 data-residual matmul.
        taps_h = [[wh[d, j] for d in range(3)] for j in range(3)]
        return _band(taps, 6, ml_dtypes.bfloat16), _band(taps_h, 3, np.float16)
    taps = [[w[d, j] for d in range(3)] for j in range(3)]
    return _band(taps, 3, np.float32), None


# ------------------------------------------------------------- bass program
def build_nc(h_out=H_CORE, wc=WC, mm_mode=MM_MODE):
    tail_op = _register_tail_op()
    h_in = h_out + 2
    nc = bacc.Bacc(None, target_bir_lowering=False)
    # const AP for the sqrt bias (activation converts float bias to an AP)
    _bias = nc.alloc_sbuf_tensor("const-sqrt-bias", [128, 1], F32)
    nc.gpsimd.memset(_bias.ap(), SQRT_BIAS)
    nc.const_aps.aps[(F32, SQRT_BIAS)] = _bias.ap()
    nc.all_engine_barrier()

    n_seg = 6 if mm_mode == "bf16x3" else 3
    w_dt = BF16 if mm_mode == "bf16x3" else F32
    mm_dt = {"bf16x3": BF16, "fp32r": F32R, "fp32": F32}[mm_mode]
    at_dt = F32 if mm_mode == "bf16x3" else mm_dt

    x = nc.declare_dram_parameter("x", [h_in, wc], F32, isOutput=False)
    wmat = nc.declare_dram_parameter("wm", [128, n_seg * BLOCK], w_dt, isOutput=False)
    if mm_mode == "bf16x3":
        wmath = nc.declare_dram_parameter("wmh", [128, 3 * BLOCK], FP16, isOutput=False)
    out = nc.declare_dram_parameter("out", [h_out, wc], F32, isOutput=True)

    # equal blocks (uniform matmul tile_size)
    n_blk = -(-h_out // BLOCK)
    base, rem = divmod(h_out, n_blk)
    blocks = []
    r0 = 0
    for i in range(n_blk):
        m = base + (1 if i < rem else 0)
        blocks.append((r0, m))
        r0 += m

    with tile.TileContext(nc) as tc:
        with (
            tc.tile_pool(name="consts", bufs=1) as cpool,
            tc.tile_pool(name="at", bufs=2) as atpool,
            tc.tile_pool(name="xpool", bufs=6) as xpool,
            tc.tile_pool(name="hpool", bufs=2) as hpool,
            tc.tile_pool(name="lpool", bufs=2) as lpool,
            tc.tile_pool(name="upool", bufs=2) as upool,
            tc.tile_pool(name="rpool", bufs=2) as rpool,
            tc.tile_pool(name="opool", bufs=3) as opool,
            tc.tile_pool(name="psum", bufs=2, space="PSUM") as pspool,
        ):
            if mm_mode in ("fp32r", "fp32"):
                # funnel weight deps through ACT (fp32 LDW has few wait slots)
                wt0 = cpool.tile([128, n_seg * BLOCK], F32)
                wt = cpool.tile([128, n_seg * BLOCK], mm_dt)
                nc.sync.dma_start(wt0[:], wmat[:])
                nc.scalar.activation(wt[:], wt0[:],
                                     mybir.ActivationFunctionType.Copy,
                                     bias=0.0, scale=1.0)
            else:
                wt = cpool.tile([128, n_seg * BLOCK], w_dt)
                nc.sync.dma_start(wt[:], wmat[:])
                wth = cpool.tile([128, 3 * BLOCK], FP16)
                nc.sync.dma_start(wth[:], wmath[:])

            for (r0, m) in blocks:
                k_in = m + 2
                at = atpool.tile([128, wc + 6], at_dt, tag="at")
                # zero 3-col borders (ACT writes -> valid fp32r producer too)
                for sl in (slice(0, 3), slice(wc + 3, wc + 6)):
                    nc.scalar.activation(at[:k_in, sl],
                                         nc.const_aps.tensor(0.0, (k_in, 3)),
                                         mybir.ActivationFunctionType.Copy,
                                         bias=0.0, scale=1.0)
                for g0 in range(0, wc, GROUP):
                    gw = min(GROUP, wc - g0)
                    xc = xpool.tile([128, GROUP], F32, tag="xc")
                    nc.sync.dma_start(xc[:k_in, :gw], x[r0:r0 + k_in, g0:g0 + gw])
                    nc.scalar.activation(at[:k_in, 3 + g0:3 + g0 + gw],
                                         xc[:k_in, :gw],
                                         mybir.ActivationFunctionType.Sqrt,
                                         bias=SQRT_BIAS, scale=SQRT_SCALE)
                for g0 in range(0, wc, GROUP):
                    gw = min(GROUP, wc - g0)
                    ps = pspool.tile([126, GROUP], F32, tag="ps")
                    if mm_mode == "bf16x3":
                        hh = hpool.tile([128, GROUP + 6], BF16, tag="hh")
                        ll = lpool.tile([128, GROUP + 6], FP16, tag="ll")
                        SPLIT_ENG = nc.gpsimd if SPLIT_ON_GPSIMD else nc.vector
                        SPLIT_ENG.tensor_copy(hh[:k_in, :gw + 6],
                                              at[:k_in, g0:g0 + gw + 6])
                        SPLIT_ENG.tensor_sub(ll[:k_in, :gw + 6],
                                             at[:k_in, g0:g0 + gw + 6],
                                             hh[:k_in, :gw + 6])
                        terms = [(0, wt, hh), (3, wt, hh), (0, wth, ll)]
                    else:
                        terms = [(0, wt, at)]
                    n_terms = 3 * len(terms)
                    for n0 in range(0, gw, CHUNK):
                        cw = min(CHUNK, gw - n0)
                        t = 0
                        for j in range(3):  # taps dx=-1,0,+1 (3*j col offset)
                            for (seg_off, wtile, rhs) in terms:
                                col = (n0 if rhs is not at else g0 + n0) + 3 * j
                                seg = (seg_off + j) * BLOCK
                                nc.tensor.matmul(
                                    ps[:m, n0:n0 + cw],
                                    wtile[:k_in, seg:seg + m],
                                    rhs[:k_in, col:col + cw],
                                    start=(t == 0), stop=(t == n_terms - 1),
                                )
                                t += 1
                    u = upool.tile([126, GROUP], F32, tag="u")
                    r = rpool.tile([126, GROUP], F32, tag="r")
                    o = opool.tile([126, GROUP], F32, tag="o")
                    nc.scalar.activation(u[:m, :gw], ps[:m, :gw],
                                         mybir.ActivationFunctionType.Square,
                                         scale=SQ_SCALE)
                    nc.vector.reciprocal_approx_fast(out=r[:m, :gw], in_=ps[:m, :gw])
                    nc.vector._custom_dve(tail_op, out=o[:m, :gw],
                                          in0=r[:m, :gw], in1=u[:m, :gw],
                                          s0=CT, s1=BT, imm2=-0.125)
                    nc.gpsimd.dma_start(out[r0:r0 + m, g0:g0 + gw], o[:m, :gw])
    nc.compile()
    return nc


# ------------------------------------------------------------------- driver
_CACHE = {}


def _get_nc(h_out, wc, mm_mode):
    key = (h_out, wc, mm_mode)
    if key not in _CACHE:
        _CACHE[key] = build_nc(h_out, wc, mm_mode)
    return _CACHE[key]


def run_sharded(x2d, n_cores=N_CORES, mm_mode=MM_MODE, trace=False, **kw):
    """x2d: [H, W*C] fp32 full image (2D). Returns ([H, W*C] fp32, results)."""
    h, wc = x2d.shape
    h_core = h // n_cores
    nc = _get_nc(h_core, wc, mm_mode)
    wm, wmh = _weight_matrices(mm_mode)
    in_maps = []
    for i in range(n_cores):
        lo, hi = i * h_core - 1, (i + 1) * h_core + 1
        src_lo, src_hi = max(lo, 0), min(hi, h)
        if lo < 0 or hi > h:
            slab = np.full((h_core + 2, wc), PAD_VAL, dtype=np.float32)
        else:
            slab = np.empty((h_core + 2, wc), dtype=np.float32)
        slab[src_lo - lo:src_hi - lo] = x2d[src_lo:src_hi]
        im_map = {"x": slab, "wm": wm}
        if wmh is not None:
            im_map["wmh"] = wmh
        in_maps.append(im_map)
    res = run_bass_kernel_spmd(nc, in_maps, list(range(n_cores)), trace=trace, **kw)
    full = np.concatenate([res.results[i]["out"] for i in range(n_cores)], axis=0)
    return full, res


def kernel(im: np.ndarray) -> np.ndarray:
    x2d = np.asarray(im, dtype=np.float32).reshape(H, WC)
    full, _ = run_sharded(x2d)
    return full.reshape(H, W, CH)
